# revision 11
# baseline (speedup 1.0000x reference)
# Trainium2 Bass kernel: depthwise 3D Gaussian low-pass filter (5x5x5, separable)
# image [4, 64, 64, 64, 32] (n, x, y, z, c) -> same-shape output, stride 1, pad 2.
#
# Sharding: 8 cores = (n, z-half). Each core owns n = k//2, z in [32*(k%2), +32)
# plus a 2-deep z halo each side (zero-padded at volume edges). All I/O fp16.
#
# Per core (partitions p = (y-parity a, x), free = (z, c)):
#   stage A (z-conv, 5-tap, runs FIRST so it consumes the z halo): per y-pair
#     block b, zt[b][p, (z32 c)] = sum_j w_j x[b][p, (z+j, c)], decomposed as
#     symmetric window-adds + scalar muls + combines (2x/4x fp16 DVE modes).
#     Work is spread across engines per 2-block unit (see _ZUNITS): DVE,
#     GpSimd adds or muls, ACT muls, or PE (5 shifted scaled-identity
#     matmuls + ACT evac).  GpSimd only gets tensor_tensor / tensor_scalar
#     (walrus rejects TensorScalarPtr-with-3-inputs on Pool).
#   stage B (xy-conv): out_block b = sum_d W_d.T @ zt[b+d], d in {-1,0,1},
#     where W_d[128,128] = Toeplitz_x (5-tap) x y-parity band; 3 fp16 matmuls
#     per 512-col chunk accumulated in fp32 PSUM, one ACT evac per block.
# DMAs are batched 4 blocks per transfer to amortize DGE overhead.
import numpy as np

_SIGMA = 0.5 * (2.0 ** 2 - 1) ** 0.5  # scale = 2.0
_KS = 5
_NC = 8
_X, _Y, _Z, _C, _NB = 64, 64, 64, 32, 4
_ZH = _Z // 2          # z extent per core (32)
_ZP = _ZH + 4          # with halo (36)
_FIN = _ZP * _C        # 1152 free elements in per block
_FOUT = _ZH * _C       # 1024 free elements out per block
_NBLK = _Y // 2        # 32 y-pair blocks
_SC = 4                # blocks per DMA super-chunk
_NSC = _NBLK // _SC    # 8 super-chunks

# Per-unit (2 consecutive y-pair blocks) z-conv engine assignment, found by
# randomized search against the instruction cost model:
#   P = PE shifted-identity matmuls (+ ACT evac)
#   K = GpSimd window-adds + DVE muls/combines
#   Q = DVE adds/combines + GpSimd muls
#   S = DVE adds/combines + ACT muls
#   D = all DVE
#   F = fast-start unit (per-block minimal-latency chains)
#   A = all nc.any (scheduler gap-fills DVE/ACT)
_ZUNITS = list("DPDPFPKPDPDKKQDQ")
assert len(_ZUNITS) == _NBLK // 2
_XBUFS = 5    # input super-chunk buffers
_ZTBUFS = 14  # zt tile buffers
_PXYBUFS = 3  # xy psum buffers
_PZBUFS = 1   # z-conv psum buffers (P units)
_TMPBUFS = 3  # z-conv scratch buffers per tag
_WARMUP = 16  # dummy matmuls to ramp the PE clock during initial DMA

_CACHE = {}


def _wn():
    r = np.arange(_KS, dtype=np.float64) - _KS // 2
    w = np.exp(-(r ** 2) / (2 * _SIGMA ** 2))
    return w / w.sum()


def _build_consts():
    wn = _wn()
    Bx = np.zeros((64, 64))
    for x in range(64):
        for xp in range(max(0, x - 2), min(64, x + 3)):
            Bx[x, xp] = wn[xp - x + 2]
    wmat = np.zeros((3, 128, 128))
    for di, d in enumerate((-1, 0, 1)):
        for a in range(2):
            for a2 in range(2):
                idx = 2 * d + a - a2 + 2
                if 0 <= idx < _KS:
                    wmat[di, a * 64:(a + 1) * 64, a2 * 64:(a2 + 1) * 64] = Bx * wn[idx]
    zmat = np.zeros((_KS, 128, 128))
    for j in range(_KS):
        zmat[j] = np.eye(128) * wn[j]
    return wmat.astype(np.float16), zmat.astype(np.float16)


def _build_nc():
    import concourse.bacc as bacc
    import concourse.mybir as mybir
    import concourse.tile as tile

    f32 = mybir.dt.float32
    f16 = mybir.dt.float16
    ADD = mybir.AluOpType.add
    MUL = mybir.AluOpType.mult

    wnf = _wn()

    nc = bacc.Bacc("TRN2", target_bir_lowering=False, debug=False,
                   num_devices=_NC)
    xin = nc.dram_tensor("xin", [_X, _Y, _ZP, _C], f16, kind="ExternalInput")
    cm = nc.dram_tensor("cm", [(3 + _KS) * 128, 128], f16, kind="ExternalInput")
    yout = nc.dram_tensor("yout", [_X, _Y, _ZH, _C], f16, kind="ExternalOutput")

    # [x, y, z, c] -> [s, a, x, (f z c)]: y = 2*(4s+f) + a.  Partition dim is
    # x (64); the two y parities need separate DMAs (a=0 -> partitions 0:64,
    # a=1 -> 64:128) since (a x) strides can't fuse into one AP dim.
    xin_v = xin.ap().rearrange("x (s f a) z c -> s a x f (z c)", f=_SC, a=2)
    yout_v = yout.ap().rearrange("x (s f a) z c -> s a x f (z c)", f=_SC, a=2)

    with tile.TileContext(nc) as tc:
        with (
            tc.tile_pool(name="consts", bufs=1) as cpool,
            tc.tile_pool(name="xsc", bufs=_XBUFS) as xpool,
            tc.tile_pool(name="zt", bufs=_ZTBUFS) as ztpool,
            tc.tile_pool(name="tmp", bufs=_TMPBUFS) as tpool,
            tc.tile_pool(name="osc", bufs=3) as opool,
            tc.tile_pool(name="pxy", bufs=_PXYBUFS, space="PSUM") as pxypool,
            tc.tile_pool(name="pz", bufs=_PZBUFS, space="PSUM") as pzpool,
        ):
            xsc = {}   # super-chunk s -> input tile [128, 4*1152]
            zt = {}    # block b -> z-convolved tile [128, 1024]
            osc = {}   # super-chunk s -> output tile [128, 4*1024]

            def load_sc(s, split=False):
                t = xpool.tile([128, _SC * _FIN], f16, tag="xsc")
                tv = t[:].rearrange("p (f q) -> p f q", f=_SC)
                if split:
                    # First chunk: land blocks 0-1 first so the z-conv
                    # pipeline starts ~1.6us earlier.
                    h = _SC // 2
                    nc.sync.dma_start(out=tv[0:64, 0:h], in_=xin_v[s, 0][:, 0:h])
                    nc.sync.dma_start(out=tv[64:128, 0:h],
                                      in_=xin_v[s, 1][:, 0:h])
                    nc.sync.dma_start(out=tv[0:64, h:_SC],
                                      in_=xin_v[s, 0][:, h:_SC])
                    nc.sync.dma_start(out=tv[64:128, h:_SC],
                                      in_=xin_v[s, 1][:, h:_SC])
                else:
                    nc.sync.dma_start(out=tv[0:64], in_=xin_v[s, 0])
                    nc.sync.dma_start(out=tv[64:128], in_=xin_v[s, 1])
                xsc[s] = t

            # All 8 [128,128] weight matrices in one DMA: dram row t*128+p
            # -> sbuf partition p, free column t*128+f.
            cmt = cpool.tile([128, (3 + _KS) * 128], f16, tag="cmt")
            nc.sync.dma_start(
                out=cmt[:].rearrange("p (t f) -> p t f", t=3 + _KS),
                in_=cm.ap().rearrange("(t p) f -> p t f", t=3 + _KS))
            load_sc(0)
            Wt = [cmt[:, di * 128:(di + 1) * 128] for di in range(3)]
            Zt = [cmt[:, (3 + j) * 128:(4 + j) * 128] for j in range(_KS)]

            w0, w1, w2 = float(wnf[0]), float(wnf[1]), float(wnf[2])

            if _WARMUP:
                # Ramp the PE clock while the first input DMA is in flight;
                # results are discarded (overwritten by the first P-unit).
                pw = pzpool.tile([128, _FOUT], f32, tag="pz", name="pz")
                for _ in range(_WARMUP):
                    nc.tensor.matmul(pw[:, 0:128], Zt[0], cmt[:, 0:128],
                                     start=True, stop=True)

            def zconv_unit(u):
                # z-conv for blocks 2u, 2u+1 as [128, 2, 1024] batched ops
                kind = _ZUNITS[u]
                b0 = 2 * u
                x = xsc[b0 // _SC]
                xv = x[:].rearrange("p (f q) -> p f q", f=_SC)
                fu = (b0 % _SC)  # first block's index within the super-chunk

                def xs(j):
                    # tap-j shifted [128, 2, 1024] view of the unit's input
                    return xv[:, fu:fu + 2, j * _C: j * _C + _FOUT]

                o = ztpool.tile([128, 2 * _FOUT], f16, tag="zt")
                ov = o[:].rearrange("p (f q) -> p f q", f=2)
                zt[b0] = o[:, 0:_FOUT]
                zt[b0 + 1] = o[:, _FOUT:2 * _FOUT]
                if kind == "P":
                    for i in range(2):
                        p = pzpool.tile([128, _FOUT], f32, tag="pz")
                        for ch in range(2):
                            co = ch * 512
                            for j in range(_KS):
                                nc.tensor.matmul(
                                    p[:, co:co + 512], Zt[j],
                                    xv[:, fu + i, j * _C + co: j * _C + co + 512],
                                    start=(j == 0), stop=(j == _KS - 1))
                        nc.scalar.copy(ov[:, i], p[:])
                elif kind == "F":
                    # Fast-start unit: fully per-block, minimal-latency DVE
                    # chain for block 0; block 1 gets GpSimd window-adds so
                    # both zts land ~simultaneously right after the first DMA.
                    for i in range(2):
                        p1 = tpool.tile([128, _FOUT], f16, tag="p1", name="p1")
                        p2 = tpool.tile([128, _FOUT], f16, tag="p2", name="p2")
                        m0 = tpool.tile([128, _FOUT], f16, tag="m0", name="m0")
                        m1 = tpool.tile([128, _FOUT], f16, tag="m1", name="m1")
                        t = tpool.tile([128, _FOUT], f16, tag="tt", name="tt")
                        lo = (fu + i) * _FIN
                        def xsl(j, lo=lo):
                            return x[:, lo + j * _C: lo + j * _C + _FOUT]
                        eng = nc.vector if i == 0 else nc.gpsimd
                        eng.tensor_tensor(p1[:], xsl(1), xsl(3), ADD)
                        eng.tensor_tensor(p2[:], xsl(0), xsl(4), ADD)
                        nc.vector.tensor_scalar_mul(m0[:], xsl(2), w2)
                        nc.vector.tensor_scalar_mul(m1[:], p1[:], w1)
                        nc.vector.tensor_tensor(t[:], m0[:], m1[:], ADD)
                        nc.vector.tensor_scalar_mul(m1[:], p2[:], w0)
                        nc.vector.tensor_tensor(ov[:, i], t[:], m1[:], ADD)
                elif kind == "K":
                    # GpSimd: window-adds p1, p2 (per block, 1D APs);
                    # DVE: muls + combines (batched)
                    p1t = tpool.tile([128, 2 * _FOUT], f16, tag="p1", name="p1")
                    p2t = tpool.tile([128, 2 * _FOUT], f16, tag="p2", name="p2")
                    m0t = tpool.tile([128, 2 * _FOUT], f16, tag="m0", name="m0")
                    m1t = tpool.tile([128, 2 * _FOUT], f16, tag="m1", name="m1")
                    ttt = tpool.tile([128, 2 * _FOUT], f16, tag="tt", name="tt")
                    for i in range(2):
                        lo = (fu + i) * _FIN
                        sl = slice(i * _FOUT, (i + 1) * _FOUT)
                        nc.gpsimd.tensor_tensor(
                            p1t[:, sl], x[:, lo + _C:lo + _C + _FOUT],
                            x[:, lo + 3 * _C:lo + 3 * _C + _FOUT], ADD)
                        nc.gpsimd.tensor_tensor(
                            p2t[:, sl], x[:, lo:lo + _FOUT],
                            x[:, lo + 4 * _C:lo + 4 * _C + _FOUT], ADD)
                    p1 = p1t[:].rearrange("p (f q) -> p f q", f=2)
                    p2 = p2t[:].rearrange("p (f q) -> p f q", f=2)
                    m0 = m0t[:].rearrange("p (f q) -> p f q", f=2)
                    m1 = m1t[:].rearrange("p (f q) -> p f q", f=2)
                    t = ttt[:].rearrange("p (f q) -> p f q", f=2)
                    nc.vector.tensor_scalar_mul(m0, xs(2), w2)
                    nc.vector.tensor_scalar_mul(m1, p1, w1)
                    nc.vector.tensor_tensor(t, m0, m1, ADD)
                    m2 = m0  # reuse
                    nc.vector.tensor_scalar_mul(m2, p2, w0)
                    nc.vector.tensor_tensor(ov, t, m2, ADD)
                else:  # Q / S / D / A: adds batched, muls+combines per block
                    p1t = tpool.tile([128, 2 * _FOUT], f16, tag="p1", name="p1")
                    p2t = tpool.tile([128, 2 * _FOUT], f16, tag="p2", name="p2")
                    p1 = p1t[:].rearrange("p (f q) -> p f q", f=2)
                    p2 = p2t[:].rearrange("p (f q) -> p f q", f=2)
                    vtt = nc.any.tensor_tensor if kind == "A" else \
                        nc.vector.tensor_tensor
                    vtt(p1, xs(1), xs(3), ADD)
                    vtt(p2, xs(0), xs(4), ADD)
                    for i in range(2):
                        m0 = tpool.tile([128, _FOUT], f16, tag="m0", name="m0")
                        m1 = tpool.tile([128, _FOUT], f16, tag="m1", name="m1")
                        m2 = tpool.tile([128, _FOUT], f16, tag="m2", name="m2")
                        t = tpool.tile([128, _FOUT], f16, tag="tt", name="tt")
                        x0 = xv[:, fu + i, 2 * _C: 2 * _C + _FOUT]
                        if kind == "Q":
                            nc.gpsimd.tensor_scalar_mul(m0[:], x0, w2)
                            nc.gpsimd.tensor_scalar_mul(m1[:], p1[:, i], w1)
                            nc.gpsimd.tensor_scalar_mul(m2[:], p2[:, i], w0)
                        elif kind == "S":
                            nc.scalar.mul(m0[:], x0, w2)
                            nc.scalar.mul(m1[:], p1[:, i], w1)
                            nc.scalar.mul(m2[:], p2[:, i], w0)
                        elif kind == "A":
                            nc.any.tensor_scalar_mul(m0[:], x0, w2)
                            nc.any.tensor_scalar_mul(m1[:], p1[:, i], w1)
                            nc.any.tensor_scalar_mul(m2[:], p2[:, i], w0)
                        else:
                            nc.vector.tensor_scalar_mul(m0[:], x0, w2)
                            nc.vector.tensor_scalar_mul(m1[:], p1[:, i], w1)
                            nc.vector.tensor_scalar_mul(m2[:], p2[:, i], w0)
                        vtt(t[:], m0[:], m1[:], ADD)
                        vtt(ov[:, i], t[:], m2[:], ADD)

            def xyconv(b):
                s = b // _SC
                if s not in osc:
                    osc[s] = opool.tile([128, _SC * _FOUT], f16, tag="osc",
                                        name="osc")
                p = pxypool.tile([128, _FOUT], f32, tag="pxy")
                ds = [d for d in (-1, 0, 1) if 0 <= b + d < _NBLK]
                for ch in range(2):
                    co = ch * 512
                    for i, d in enumerate(ds):
                        nc.tensor.matmul(
                            p[:, co:co + 512], Wt[d + 1],
                            zt[b + d][:, co:co + 512],
                            start=(i == 0), stop=(i == len(ds) - 1))
                lo = (b % _SC) * _FOUT
                nc.scalar.copy(osc[s][:, lo:lo + _FOUT], p[:])
                if b % 2 == 1:
                    # Output DMA (half a super-chunk at a time) on the ACT
                    # DGE queue, emitted right after the evac of its second
                    # block (same engine, so the wait is already satisfied at
                    # dispatch and never blocks the input queue on SP).
                    ov = osc[s][:].rearrange("p (f q) -> p f q", f=_SC)
                    f0 = (b % _SC) - 1
                    nc.scalar.dma_start(out=yout_v[s, 0][:, f0:f0 + 2],
                                        in_=ov[0:64, f0:f0 + 2])
                    nc.scalar.dma_start(out=yout_v[s, 1][:, f0:f0 + 2],
                                        in_=ov[64:128, f0:f0 + 2])

            next_xy = 0
            for s in range(_NSC):
                if s > 0:
                    load_sc(s)
                for u in range(s * _SC // 2, (s + 1) * _SC // 2):
                    zconv_unit(u)
                # xy for blocks whose zt[b+1] now exists
                hi = min(s * _SC + _SC - 2, _NBLK - 1) if s < _NSC - 1 \
                    else _NBLK - 1
                while next_xy <= hi:
                    xyconv(next_xy)
                    next_xy += 1
    nc.compile()
    return nc


def kernel(image, kernel, _trace=False):
    from concourse.bass_utils import run_bass_kernel_spmd

    image = np.asarray(image)
    if "nc" not in _CACHE:
        _CACHE["nc"] = _build_nc()
        _CACHE["consts"] = _build_consts()
    nc = _CACHE["nc"]
    wmat, zmat = _CACHE["consts"]
    cm = np.concatenate([wmat.reshape(3 * 128, 128),
                         zmat.reshape(_KS * 128, 128)], axis=0)
    cm = np.ascontiguousarray(cm, dtype=np.float16)

    img16 = np.ascontiguousarray(image, dtype=np.float16)
    in_maps = []
    for k in range(_NC):
        n, h = k // 2, k % 2
        zlo = h * _ZH - 2
        xin = np.zeros((_X, _Y, _ZP, _C), np.float16)
        s0, s1 = max(0, zlo), min(_Z, zlo + _ZP)
        xin[:, :, s0 - zlo: s1 - zlo, :] = img16[n, :, :, s0:s1, :]
        in_maps.append({"xin": xin, "cm": cm})

    res = run_bass_kernel_spmd(nc, in_maps, list(range(_NC)), trace=_trace)
    out = np.empty((_NB, _X, _Y, _Z, _C), np.float32)
    for k in range(_NC):
        n, h = k // 2, k % 2
        out[n, :, :, h * _ZH:(h + 1) * _ZH, :] = \
            res.results[k]["yout"].astype(np.float32)
    if _trace:
        return out, res
    return out


# revision 14
# speedup vs baseline: 1.0782x; 1.0782x over previous
# Trainium2 Bass kernel: depthwise 3D Gaussian low-pass filter (5x5x5, separable)
# image [4, 64, 64, 64, 32] (n, x, y, z, c) -> same-shape output, stride 1, pad 2.
#
# Sharding: 8 cores = (n, z-half). Each core owns n = k//2, z in [32*(k%2), +32)
# plus a 2-deep z halo each side (zero-padded at volume edges). All I/O fp16.
#
# Per core (partitions p = (y-parity a, x), free = (z, c)):
#   stage A (z-conv, 5-tap, runs FIRST so it consumes the z halo): per y-pair
#     block b, zt[b][p, (z32 c)] = sum_j w_j x[b][p, (z+j, c)], decomposed as
#     symmetric window-adds + scalar muls + combines (2x/4x fp16 DVE modes).
#     Work is spread across engines per 2-block unit (see _ZUNITS): DVE,
#     GpSimd adds or muls, ACT muls, or PE (5 shifted scaled-identity
#     matmuls + ACT evac).  GpSimd only gets tensor_tensor / tensor_scalar
#     (walrus rejects TensorScalarPtr-with-3-inputs on Pool).
#   stage B (xy-conv): out_block b = sum_d W_d.T @ zt[b+d], d in {-1,0,1},
#     where W_d[128,128] = Toeplitz_x (5-tap) x y-parity band; 3 fp16 matmuls
#     per 512-col chunk accumulated in fp32 PSUM, one ACT evac per block.
# DMAs are batched 4 blocks per transfer to amortize DGE overhead.
import numpy as np

_SIGMA = 0.5 * (2.0 ** 2 - 1) ** 0.5  # scale = 2.0
_KS = 5
_NC = 8
_X, _Y, _Z, _C, _NB = 64, 64, 64, 32, 4
_ZH = _Z // 2          # z extent per core (32)
_ZP = _ZH + 4          # with halo (36)
_FIN = _ZP * _C        # 1152 free elements in per block
_FOUT = _ZH * _C       # 1024 free elements out per block
_NBLK = _Y // 2        # 32 y-pair blocks
_SC = 4                # blocks per DMA super-chunk
_NSC = _NBLK // _SC    # 8 super-chunks

# Per-unit (2 consecutive y-pair blocks) z-conv engine assignment, found by
# randomized search against the instruction cost model:
#   P = PE shifted-identity matmuls (+ ACT evac)
#   K = GpSimd window-adds + DVE muls/combines
#   Q = DVE adds/combines + GpSimd muls
#   S = DVE adds/combines + ACT muls
#   D = all DVE
#   F = fast-start unit (per-block minimal-latency chains)
#   A = all nc.any (scheduler gap-fills DVE/ACT)
_ZUNITS = list("PDDPFPKPDPDKKQDQ")
assert len(_ZUNITS) == _NBLK // 2
_XBUFS = 5    # input super-chunk buffers
_ZTBUFS = 14  # zt tile buffers
_PXYBUFS = 3  # xy psum buffers
_PZBUFS = 1   # z-conv psum buffers (P units)
_TMPBUFS = 3  # z-conv scratch buffers per tag
_WARMUP = 16  # dummy matmuls to ramp the PE clock during initial DMA

_CACHE = {}


def _wn():
    r = np.arange(_KS, dtype=np.float64) - _KS // 2
    w = np.exp(-(r ** 2) / (2 * _SIGMA ** 2))
    return w / w.sum()


def _build_consts():
    wn = _wn()
    Bx = np.zeros((64, 64))
    for x in range(64):
        for xp in range(max(0, x - 2), min(64, x + 3)):
            Bx[x, xp] = wn[xp - x + 2]
    wmat = np.zeros((3, 128, 128))
    for di, d in enumerate((-1, 0, 1)):
        for a in range(2):
            for a2 in range(2):
                idx = 2 * d + a - a2 + 2
                if 0 <= idx < _KS:
                    wmat[di, a * 64:(a + 1) * 64, a2 * 64:(a2 + 1) * 64] = Bx * wn[idx]
    zmat = np.zeros((_KS, 128, 128))
    for j in range(_KS):
        zmat[j] = np.eye(128) * wn[j]
    return wmat.astype(np.float16), zmat.astype(np.float16)


def _build_nc():
    import concourse.bacc as bacc
    import concourse.mybir as mybir
    import concourse.tile as tile

    f32 = mybir.dt.float32
    f16 = mybir.dt.float16
    ADD = mybir.AluOpType.add
    MUL = mybir.AluOpType.mult

    wnf = _wn()

    nc = bacc.Bacc("TRN2", target_bir_lowering=False, debug=False,
                   num_devices=_NC)
    xin = nc.dram_tensor("xin", [_X, _Y, _ZP, _C], f16, kind="ExternalInput")
    cm = nc.dram_tensor("cm", [(3 + _KS) * 128, 128], f16, kind="ExternalInput")
    yout = nc.dram_tensor("yout", [_X, _Y, _ZH, _C], f16, kind="ExternalOutput")

    # [x, y, z, c] -> [s, a, x, (f z c)]: y = 2*(4s+f) + a.  Partition dim is
    # x (64); the two y parities need separate DMAs (a=0 -> partitions 0:64,
    # a=1 -> 64:128) since (a x) strides can't fuse into one AP dim.
    xin_v = xin.ap().rearrange("x (s f a) z c -> s a x f (z c)", f=_SC, a=2)
    yout_v = yout.ap().rearrange("x (s f a) z c -> s a x f (z c)", f=_SC, a=2)

    with tile.TileContext(nc) as tc:
        with (
            tc.tile_pool(name="consts", bufs=1) as cpool,
            tc.tile_pool(name="xsc", bufs=_XBUFS) as xpool,
            tc.tile_pool(name="zt", bufs=_ZTBUFS) as ztpool,
            tc.tile_pool(name="tmp", bufs=_TMPBUFS) as tpool,
            tc.tile_pool(name="osc", bufs=3) as opool,
            tc.tile_pool(name="pxy", bufs=_PXYBUFS, space="PSUM") as pxypool,
            tc.tile_pool(name="pz", bufs=_PZBUFS, space="PSUM") as pzpool,
        ):
            xsc = {}   # super-chunk s -> input tile [128, 4*1152]
            zt = {}    # block b -> z-convolved tile [128, 1024]
            osc = {}   # super-chunk s -> output tile [128, 4*1024]

            def load_sc(s, split=False):
                t = xpool.tile([128, _SC * _FIN], f16, tag="xsc")
                tv = t[:].rearrange("p (f q) -> p f q", f=_SC)
                if split:
                    # First chunk: land blocks 0-1 first so the z-conv
                    # pipeline starts ~1.6us earlier.
                    h = _SC // 2
                    nc.sync.dma_start(out=tv[0:64, 0:h], in_=xin_v[s, 0][:, 0:h])
                    nc.sync.dma_start(out=tv[64:128, 0:h],
                                      in_=xin_v[s, 1][:, 0:h])
                    nc.sync.dma_start(out=tv[0:64, h:_SC],
                                      in_=xin_v[s, 0][:, h:_SC])
                    nc.sync.dma_start(out=tv[64:128, h:_SC],
                                      in_=xin_v[s, 1][:, h:_SC])
                else:
                    nc.sync.dma_start(out=tv[0:64], in_=xin_v[s, 0])
                    nc.sync.dma_start(out=tv[64:128], in_=xin_v[s, 1])
                xsc[s] = t

            # All 8 [128,128] weight matrices in one DMA: dram row t*128+p
            # -> sbuf partition p, free column t*128+f.
            cmt = cpool.tile([128, (3 + _KS) * 128], f16, tag="cmt")
            nc.sync.dma_start(
                out=cmt[:].rearrange("p (t f) -> p t f", t=3 + _KS),
                in_=cm.ap().rearrange("(t p) f -> p t f", t=3 + _KS))
            load_sc(0)
            Wt = [cmt[:, di * 128:(di + 1) * 128] for di in range(3)]
            Zt = [cmt[:, (3 + j) * 128:(4 + j) * 128] for j in range(_KS)]

            w0, w1, w2 = float(wnf[0]), float(wnf[1]), float(wnf[2])

            if _WARMUP:
                # Ramp the PE clock while the first input DMA is in flight;
                # results are discarded (overwritten by the first P-unit).
                pw = pzpool.tile([128, _FOUT], f32, tag="pz", name="pz")
                for _ in range(_WARMUP):
                    nc.tensor.matmul(pw[:, 0:128], Zt[0], cmt[:, 0:128],
                                     start=True, stop=True)

            def zconv_unit(u):
                # z-conv for blocks 2u, 2u+1 as [128, 2, 1024] batched ops
                kind = _ZUNITS[u]
                b0 = 2 * u
                x = xsc[b0 // _SC]
                xv = x[:].rearrange("p (f q) -> p f q", f=_SC)
                fu = (b0 % _SC)  # first block's index within the super-chunk

                def xs(j):
                    # tap-j shifted [128, 2, 1024] view of the unit's input
                    return xv[:, fu:fu + 2, j * _C: j * _C + _FOUT]

                o = ztpool.tile([128, 2 * _FOUT], f16, tag="zt")
                ov = o[:].rearrange("p (f q) -> p f q", f=2)
                zt[b0] = o[:, 0:_FOUT]
                zt[b0 + 1] = o[:, _FOUT:2 * _FOUT]
                if kind == "P":
                    for i in range(2):
                        p = pzpool.tile([128, _FOUT], f32, tag="pz")
                        for ch in range(2):
                            co = ch * 512
                            for j in range(_KS):
                                nc.tensor.matmul(
                                    p[:, co:co + 512], Zt[j],
                                    xv[:, fu + i, j * _C + co: j * _C + co + 512],
                                    start=(j == 0), stop=(j == _KS - 1))
                        nc.scalar.copy(ov[:, i], p[:])
                elif kind == "F":
                    # Fast-start unit: fully per-block, minimal-latency DVE
                    # chain for block 0; block 1 gets GpSimd window-adds so
                    # both zts land ~simultaneously right after the first DMA.
                    for i in range(2):
                        p1 = tpool.tile([128, _FOUT], f16, tag="p1", name="p1")
                        p2 = tpool.tile([128, _FOUT], f16, tag="p2", name="p2")
                        m0 = tpool.tile([128, _FOUT], f16, tag="m0", name="m0")
                        m1 = tpool.tile([128, _FOUT], f16, tag="m1", name="m1")
                        t = tpool.tile([128, _FOUT], f16, tag="tt", name="tt")
                        lo = (fu + i) * _FIN
                        def xsl(j, lo=lo):
                            return x[:, lo + j * _C: lo + j * _C + _FOUT]
                        eng = nc.vector if i == 0 else nc.gpsimd
                        eng.tensor_tensor(p1[:], xsl(1), xsl(3), ADD)
                        eng.tensor_tensor(p2[:], xsl(0), xsl(4), ADD)
                        nc.vector.tensor_scalar_mul(m0[:], xsl(2), w2)
                        nc.vector.tensor_scalar_mul(m1[:], p1[:], w1)
                        nc.vector.tensor_tensor(t[:], m0[:], m1[:], ADD)
                        nc.vector.tensor_scalar_mul(m1[:], p2[:], w0)
                        nc.vector.tensor_tensor(ov[:, i], t[:], m1[:], ADD)
                elif kind == "K":
                    # GpSimd: window-adds p1, p2 (per block, 1D APs);
                    # DVE: muls + combines (batched)
                    p1t = tpool.tile([128, 2 * _FOUT], f16, tag="p1", name="p1")
                    p2t = tpool.tile([128, 2 * _FOUT], f16, tag="p2", name="p2")
                    m0t = tpool.tile([128, 2 * _FOUT], f16, tag="m0", name="m0")
                    m1t = tpool.tile([128, 2 * _FOUT], f16, tag="m1", name="m1")
                    ttt = tpool.tile([128, 2 * _FOUT], f16, tag="tt", name="tt")
                    for i in range(2):
                        lo = (fu + i) * _FIN
                        sl = slice(i * _FOUT, (i + 1) * _FOUT)
                        nc.gpsimd.tensor_tensor(
                            p1t[:, sl], x[:, lo + _C:lo + _C + _FOUT],
                            x[:, lo + 3 * _C:lo + 3 * _C + _FOUT], ADD)
                        nc.gpsimd.tensor_tensor(
                            p2t[:, sl], x[:, lo:lo + _FOUT],
                            x[:, lo + 4 * _C:lo + 4 * _C + _FOUT], ADD)
                    p1 = p1t[:].rearrange("p (f q) -> p f q", f=2)
                    p2 = p2t[:].rearrange("p (f q) -> p f q", f=2)
                    m0 = m0t[:].rearrange("p (f q) -> p f q", f=2)
                    m1 = m1t[:].rearrange("p (f q) -> p f q", f=2)
                    t = ttt[:].rearrange("p (f q) -> p f q", f=2)
                    nc.vector.tensor_scalar_mul(m0, xs(2), w2)
                    nc.vector.tensor_scalar_mul(m1, p1, w1)
                    nc.vector.tensor_tensor(t, m0, m1, ADD)
                    m2 = m0  # reuse
                    nc.vector.tensor_scalar_mul(m2, p2, w0)
                    nc.vector.tensor_tensor(ov, t, m2, ADD)
                else:  # Q / S / D / A: adds batched, muls+combines per block
                    p1t = tpool.tile([128, 2 * _FOUT], f16, tag="p1", name="p1")
                    p2t = tpool.tile([128, 2 * _FOUT], f16, tag="p2", name="p2")
                    p1 = p1t[:].rearrange("p (f q) -> p f q", f=2)
                    p2 = p2t[:].rearrange("p (f q) -> p f q", f=2)
                    vtt = nc.any.tensor_tensor if kind == "A" else \
                        nc.vector.tensor_tensor
                    vtt(p1, xs(1), xs(3), ADD)
                    vtt(p2, xs(0), xs(4), ADD)
                    for i in range(2):
                        m0 = tpool.tile([128, _FOUT], f16, tag="m0", name="m0")
                        m1 = tpool.tile([128, _FOUT], f16, tag="m1", name="m1")
                        m2 = tpool.tile([128, _FOUT], f16, tag="m2", name="m2")
                        t = tpool.tile([128, _FOUT], f16, tag="tt", name="tt")
                        x0 = xv[:, fu + i, 2 * _C: 2 * _C + _FOUT]
                        if kind == "Q":
                            nc.gpsimd.tensor_scalar_mul(m0[:], x0, w2)
                            nc.gpsimd.tensor_scalar_mul(m1[:], p1[:, i], w1)
                            nc.gpsimd.tensor_scalar_mul(m2[:], p2[:, i], w0)
                        elif kind == "S":
                            nc.scalar.mul(m0[:], x0, w2)
                            nc.scalar.mul(m1[:], p1[:, i], w1)
                            nc.scalar.mul(m2[:], p2[:, i], w0)
                        elif kind == "A":
                            nc.any.tensor_scalar_mul(m0[:], x0, w2)
                            nc.any.tensor_scalar_mul(m1[:], p1[:, i], w1)
                            nc.any.tensor_scalar_mul(m2[:], p2[:, i], w0)
                        else:
                            nc.vector.tensor_scalar_mul(m0[:], x0, w2)
                            nc.vector.tensor_scalar_mul(m1[:], p1[:, i], w1)
                            nc.vector.tensor_scalar_mul(m2[:], p2[:, i], w0)
                        vtt(t[:], m0[:], m1[:], ADD)
                        vtt(ov[:, i], t[:], m2[:], ADD)

            def xyconv(b):
                s = b // _SC
                if s not in osc:
                    osc[s] = opool.tile([128, _SC * _FOUT], f16, tag="osc",
                                        name="osc")
                p = pxypool.tile([128, _FOUT], f32, tag="pxy")
                ds = [d for d in (-1, 0, 1) if 0 <= b + d < _NBLK]
                for ch in range(2):
                    co = ch * 512
                    for i, d in enumerate(ds):
                        nc.tensor.matmul(
                            p[:, co:co + 512], Wt[d + 1],
                            zt[b + d][:, co:co + 512],
                            start=(i == 0), stop=(i == len(ds) - 1))
                lo = (b % _SC) * _FOUT
                nc.scalar.copy(osc[s][:, lo:lo + _FOUT], p[:])
                # Output DMAs ride the ACT DGE queue (half a super-chunk at
                # a time), emitted right after the evac of their second
                # block (same engine, so the wait is already satisfied at
                # dispatch and never blocks the input queue on SP).
                ov = osc[s][:].rearrange("p (f q) -> p f q", f=_SC)
                if b % 2 == 1:
                    f0 = (b % _SC) - 1
                    nc.scalar.dma_start(out=yout_v[s, 0][:, f0:f0 + 2],
                                        in_=ov[0:64, f0:f0 + 2])
                    nc.scalar.dma_start(out=yout_v[s, 1][:, f0:f0 + 2],
                                        in_=ov[64:128, f0:f0 + 2])

            next_xy = 0
            for s in range(_NSC):
                if s > 0:
                    load_sc(s)
                for u in range(s * _SC // 2, (s + 1) * _SC // 2):
                    zconv_unit(u)
                # xy for blocks whose zt[b+1] now exists
                hi = min(s * _SC + _SC - 2, _NBLK - 1) if s < _NSC - 1 \
                    else _NBLK - 1
                while next_xy <= hi:
                    xyconv(next_xy)
                    next_xy += 1
    nc.compile()
    return nc


def kernel(image, kernel, _trace=False):
    from concourse.bass_utils import run_bass_kernel_spmd

    image = np.asarray(image)
    if "nc" not in _CACHE:
        _CACHE["nc"] = _build_nc()
        _CACHE["consts"] = _build_consts()
    nc = _CACHE["nc"]
    wmat, zmat = _CACHE["consts"]
    cm = np.concatenate([wmat.reshape(3 * 128, 128),
                         zmat.reshape(_KS * 128, 128)], axis=0)
    cm = np.ascontiguousarray(cm, dtype=np.float16)

    img16 = np.ascontiguousarray(image, dtype=np.float16)
    in_maps = []
    for k in range(_NC):
        n, h = k // 2, k % 2
        zlo = h * _ZH - 2
        xin = np.zeros((_X, _Y, _ZP, _C), np.float16)
        s0, s1 = max(0, zlo), min(_Z, zlo + _ZP)
        xin[:, :, s0 - zlo: s1 - zlo, :] = img16[n, :, :, s0:s1, :]
        in_maps.append({"xin": xin, "cm": cm})

    res = run_bass_kernel_spmd(nc, in_maps, list(range(_NC)), trace=_trace)
    out = np.empty((_NB, _X, _Y, _Z, _C), np.float32)
    for k in range(_NC):
        n, h = k // 2, k % 2
        out[n, :, :, h * _ZH:(h + 1) * _ZH, :] = \
            res.results[k]["yout"].astype(np.float32)
    if _trace:
        return out, res
    return out


# revision 16
# speedup vs baseline: 1.0901x; 1.0110x over previous
# Trainium2 Bass kernel: depthwise 3D Gaussian low-pass filter (5x5x5, separable)
# image [4, 64, 64, 64, 32] (n, x, y, z, c) -> same-shape output, stride 1, pad 2.
#
# Sharding: 8 cores = (n, z-half). Each core owns n = k//2, z in [32*(k%2), +32)
# plus a 2-deep z halo each side (zero-padded at volume edges). All I/O fp16.
#
# Per core (partitions p = (y-parity a, x), free = (z, c)):
#   stage A (z-conv, 5-tap, runs FIRST so it consumes the z halo): per y-pair
#     block b, zt[b][p, (z32 c)] = sum_j w_j x[b][p, (z+j, c)], decomposed as
#     symmetric window-adds + scalar muls + combines (2x/4x fp16 DVE modes).
#     Work is spread across engines per 2-block unit (see _ZUNITS): DVE,
#     GpSimd adds or muls, ACT muls, or PE (5 shifted scaled-identity
#     matmuls + ACT evac).  GpSimd only gets tensor_tensor / tensor_scalar
#     (walrus rejects TensorScalarPtr-with-3-inputs on Pool).
#   stage B (xy-conv): out_block b = sum_d W_d.T @ zt[b+d], d in {-1,0,1},
#     where W_d[128,128] = Toeplitz_x (5-tap) x y-parity band; 3 fp16 matmuls
#     per 512-col chunk accumulated in fp32 PSUM, one ACT evac per block.
# DMAs are batched 4 blocks per transfer to amortize DGE overhead.
import numpy as np

_SIGMA = 0.5 * (2.0 ** 2 - 1) ** 0.5  # scale = 2.0
_KS = 5
_NC = 8
_X, _Y, _Z, _C, _NB = 64, 64, 64, 32, 4
_ZH = _Z // 2          # z extent per core (32)
_ZP = _ZH + 4          # with halo (36)
_FIN = _ZP * _C        # 1152 free elements in per block
_FOUT = _ZH * _C       # 1024 free elements out per block
_NBLK = _Y // 2        # 32 y-pair blocks
_SC = 4                # blocks per DMA super-chunk
_NSC = _NBLK // _SC    # 8 super-chunks

# Per-unit (2 consecutive y-pair blocks) z-conv engine assignment, found by
# randomized search against the instruction cost model:
#   P = PE shifted-identity matmuls (+ ACT evac)
#   K = GpSimd window-adds + DVE muls/combines
#   Q = DVE adds/combines + GpSimd muls
#   S = DVE adds/combines + ACT muls
#   D = all DVE
#   F = fast-start unit (per-block minimal-latency chains)
#   A = all nc.any (scheduler gap-fills DVE/ACT)
_ZUNITS = list("PDDPFPKPDPDKKQDD")
assert len(_ZUNITS) == _NBLK // 2
_XBUFS = 5    # input super-chunk buffers
_ZTBUFS = 14  # zt tile buffers
_PXYBUFS = 3  # xy psum buffers
_PZBUFS = 1   # z-conv psum buffers (P units)
_TMPBUFS = 3  # z-conv scratch buffers per tag
_WARMUP = 16  # dummy matmuls to ramp the PE clock during initial DMA

_CACHE = {}


def _wn():
    r = np.arange(_KS, dtype=np.float64) - _KS // 2
    w = np.exp(-(r ** 2) / (2 * _SIGMA ** 2))
    return w / w.sum()


def _build_consts():
    wn = _wn()
    Bx = np.zeros((64, 64))
    for x in range(64):
        for xp in range(max(0, x - 2), min(64, x + 3)):
            Bx[x, xp] = wn[xp - x + 2]
    wmat = np.zeros((3, 128, 128))
    for di, d in enumerate((-1, 0, 1)):
        for a in range(2):
            for a2 in range(2):
                idx = 2 * d + a - a2 + 2
                if 0 <= idx < _KS:
                    wmat[di, a * 64:(a + 1) * 64, a2 * 64:(a2 + 1) * 64] = Bx * wn[idx]
    zmat = np.zeros((_KS, 128, 128))
    for j in range(_KS):
        zmat[j] = np.eye(128) * wn[j]
    return wmat.astype(np.float16), zmat.astype(np.float16)


def _build_nc():
    import concourse.bacc as bacc
    import concourse.mybir as mybir
    import concourse.tile as tile

    f32 = mybir.dt.float32
    f16 = mybir.dt.float16
    ADD = mybir.AluOpType.add
    MUL = mybir.AluOpType.mult

    wnf = _wn()

    nc = bacc.Bacc("TRN2", target_bir_lowering=False, debug=False,
                   num_devices=_NC)
    xin = nc.dram_tensor("xin", [_X, _Y, _ZP, _C], f16, kind="ExternalInput")
    cm = nc.dram_tensor("cm", [128, (3 + _KS) * 128], f16, kind="ExternalInput")
    yout = nc.dram_tensor("yout", [_X, _Y, _ZH, _C], f16, kind="ExternalOutput")

    # [x, y, z, c] -> [s, a, x, (f z c)]: y = 2*(4s+f) + a.  Partition dim is
    # x (64); the two y parities need separate DMAs (a=0 -> partitions 0:64,
    # a=1 -> 64:128) since (a x) strides can't fuse into one AP dim.
    xin_v = xin.ap().rearrange("x (s f a) z c -> s a x f (z c)", f=_SC, a=2)
    yout_v = yout.ap().rearrange("x (s f a) z c -> s a x f (z c)", f=_SC, a=2)

    with tile.TileContext(nc) as tc:
        with (
            tc.tile_pool(name="consts", bufs=1) as cpool,
            tc.tile_pool(name="xsc", bufs=_XBUFS) as xpool,
            tc.tile_pool(name="zt", bufs=_ZTBUFS) as ztpool,
            tc.tile_pool(name="tmp", bufs=_TMPBUFS) as tpool,
            tc.tile_pool(name="osc", bufs=3) as opool,
            tc.tile_pool(name="pxy", bufs=_PXYBUFS, space="PSUM") as pxypool,
            tc.tile_pool(name="pz", bufs=_PZBUFS, space="PSUM") as pzpool,
        ):
            xsc = {}   # super-chunk s -> input tile [128, 4*1152]
            zt = {}    # block b -> z-convolved tile [128, 1024]
            osc = {}   # super-chunk s -> output tile [128, 4*1024]

            def load_sc(s, split=False):
                t = xpool.tile([128, _SC * _FIN], f16, tag="xsc")
                tv = t[:].rearrange("p (f q) -> p f q", f=_SC)
                if split:
                    # First chunk: land blocks 0-1 first so the z-conv
                    # pipeline starts ~1.6us earlier.
                    h = _SC // 2
                    nc.sync.dma_start(out=tv[0:64, 0:h], in_=xin_v[s, 0][:, 0:h])
                    nc.sync.dma_start(out=tv[64:128, 0:h],
                                      in_=xin_v[s, 1][:, 0:h])
                    nc.sync.dma_start(out=tv[0:64, h:_SC],
                                      in_=xin_v[s, 0][:, h:_SC])
                    nc.sync.dma_start(out=tv[64:128, h:_SC],
                                      in_=xin_v[s, 1][:, h:_SC])
                else:
                    nc.sync.dma_start(out=tv[0:64], in_=xin_v[s, 0])
                    nc.sync.dma_start(out=tv[64:128], in_=xin_v[s, 1])
                xsc[s] = t

            # All 8 [128,128] weight matrices in one DMA; cm is
            # pre-transposed host-side to [p, (t f)] so the transfer is 128
            # fully-contiguous 2KB runs (sub-512B runs pay a 2x DMA penalty).
            cmt = cpool.tile([128, (3 + _KS) * 128], f16, tag="cmt")
            nc.sync.dma_start(out=cmt[:], in_=cm.ap()[:, :])
            load_sc(0)
            Wt = [cmt[:, di * 128:(di + 1) * 128] for di in range(3)]
            Zt = [cmt[:, (3 + j) * 128:(4 + j) * 128] for j in range(_KS)]

            w0, w1, w2 = float(wnf[0]), float(wnf[1]), float(wnf[2])

            if _WARMUP:
                # Ramp the PE clock while the first input DMA is in flight;
                # results are discarded (overwritten by the first P-unit).
                pw = pzpool.tile([128, _FOUT], f32, tag="pz", name="pz")
                for _ in range(_WARMUP):
                    nc.tensor.matmul(pw[:, 0:128], Zt[0], cmt[:, 0:128],
                                     start=True, stop=True)

            def zconv_unit(u):
                # z-conv for blocks 2u, 2u+1 as [128, 2, 1024] batched ops
                kind = _ZUNITS[u]
                b0 = 2 * u
                x = xsc[b0 // _SC]
                xv = x[:].rearrange("p (f q) -> p f q", f=_SC)
                fu = (b0 % _SC)  # first block's index within the super-chunk

                def xs(j):
                    # tap-j shifted [128, 2, 1024] view of the unit's input
                    return xv[:, fu:fu + 2, j * _C: j * _C + _FOUT]

                o = ztpool.tile([128, 2 * _FOUT], f16, tag="zt")
                ov = o[:].rearrange("p (f q) -> p f q", f=2)
                zt[b0] = o[:, 0:_FOUT]
                zt[b0 + 1] = o[:, _FOUT:2 * _FOUT]
                if kind == "P":
                    for i in range(2):
                        p = pzpool.tile([128, _FOUT], f32, tag="pz")
                        for ch in range(2):
                            co = ch * 512
                            for j in range(_KS):
                                nc.tensor.matmul(
                                    p[:, co:co + 512], Zt[j],
                                    xv[:, fu + i, j * _C + co: j * _C + co + 512],
                                    start=(j == 0), stop=(j == _KS - 1))
                        nc.scalar.copy(ov[:, i], p[:])
                elif kind == "F":
                    # Fast-start unit: fully per-block, minimal-latency DVE
                    # chain for block 0; block 1 gets GpSimd window-adds so
                    # both zts land ~simultaneously right after the first DMA.
                    for i in range(2):
                        p1 = tpool.tile([128, _FOUT], f16, tag="p1", name="p1")
                        p2 = tpool.tile([128, _FOUT], f16, tag="p2", name="p2")
                        m0 = tpool.tile([128, _FOUT], f16, tag="m0", name="m0")
                        m1 = tpool.tile([128, _FOUT], f16, tag="m1", name="m1")
                        t = tpool.tile([128, _FOUT], f16, tag="tt", name="tt")
                        lo = (fu + i) * _FIN
                        def xsl(j, lo=lo):
                            return x[:, lo + j * _C: lo + j * _C + _FOUT]
                        eng = nc.vector if i == 0 else nc.gpsimd
                        eng.tensor_tensor(p1[:], xsl(1), xsl(3), ADD)
                        eng.tensor_tensor(p2[:], xsl(0), xsl(4), ADD)
                        nc.vector.tensor_scalar_mul(m0[:], xsl(2), w2)
                        nc.vector.tensor_scalar_mul(m1[:], p1[:], w1)
                        nc.vector.tensor_tensor(t[:], m0[:], m1[:], ADD)
                        nc.vector.tensor_scalar_mul(m1[:], p2[:], w0)
                        nc.vector.tensor_tensor(ov[:, i], t[:], m1[:], ADD)
                elif kind == "K":
                    # GpSimd: window-adds p1, p2 (per block, 1D APs);
                    # DVE: muls + combines (batched)
                    p1t = tpool.tile([128, 2 * _FOUT], f16, tag="p1", name="p1")
                    p2t = tpool.tile([128, 2 * _FOUT], f16, tag="p2", name="p2")
                    m0t = tpool.tile([128, 2 * _FOUT], f16, tag="m0", name="m0")
                    m1t = tpool.tile([128, 2 * _FOUT], f16, tag="m1", name="m1")
                    ttt = tpool.tile([128, 2 * _FOUT], f16, tag="tt", name="tt")
                    for i in range(2):
                        lo = (fu + i) * _FIN
                        sl = slice(i * _FOUT, (i + 1) * _FOUT)
                        nc.gpsimd.tensor_tensor(
                            p1t[:, sl], x[:, lo + _C:lo + _C + _FOUT],
                            x[:, lo + 3 * _C:lo + 3 * _C + _FOUT], ADD)
                        nc.gpsimd.tensor_tensor(
                            p2t[:, sl], x[:, lo:lo + _FOUT],
                            x[:, lo + 4 * _C:lo + 4 * _C + _FOUT], ADD)
                    p1 = p1t[:].rearrange("p (f q) -> p f q", f=2)
                    p2 = p2t[:].rearrange("p (f q) -> p f q", f=2)
                    m0 = m0t[:].rearrange("p (f q) -> p f q", f=2)
                    m1 = m1t[:].rearrange("p (f q) -> p f q", f=2)
                    t = ttt[:].rearrange("p (f q) -> p f q", f=2)
                    nc.vector.tensor_scalar_mul(m0, xs(2), w2)
                    nc.vector.tensor_scalar_mul(m1, p1, w1)
                    nc.vector.tensor_tensor(t, m0, m1, ADD)
                    m2 = m0  # reuse
                    nc.vector.tensor_scalar_mul(m2, p2, w0)
                    nc.vector.tensor_tensor(ov, t, m2, ADD)
                else:  # Q / S / D / A: adds batched, muls+combines per block
                    p1t = tpool.tile([128, 2 * _FOUT], f16, tag="p1", name="p1")
                    p2t = tpool.tile([128, 2 * _FOUT], f16, tag="p2", name="p2")
                    p1 = p1t[:].rearrange("p (f q) -> p f q", f=2)
                    p2 = p2t[:].rearrange("p (f q) -> p f q", f=2)
                    vtt = nc.any.tensor_tensor if kind == "A" else \
                        nc.vector.tensor_tensor
                    vtt(p1, xs(1), xs(3), ADD)
                    vtt(p2, xs(0), xs(4), ADD)
                    for i in range(2):
                        m0 = tpool.tile([128, _FOUT], f16, tag="m0", name="m0")
                        m1 = tpool.tile([128, _FOUT], f16, tag="m1", name="m1")
                        m2 = tpool.tile([128, _FOUT], f16, tag="m2", name="m2")
                        t = tpool.tile([128, _FOUT], f16, tag="tt", name="tt")
                        x0 = xv[:, fu + i, 2 * _C: 2 * _C + _FOUT]
                        if kind == "Q":
                            nc.gpsimd.tensor_scalar_mul(m0[:], x0, w2)
                            nc.gpsimd.tensor_scalar_mul(m1[:], p1[:, i], w1)
                            nc.gpsimd.tensor_scalar_mul(m2[:], p2[:, i], w0)
                        elif kind == "S":
                            nc.scalar.mul(m0[:], x0, w2)
                            nc.scalar.mul(m1[:], p1[:, i], w1)
                            nc.scalar.mul(m2[:], p2[:, i], w0)
                        elif kind == "A":
                            nc.any.tensor_scalar_mul(m0[:], x0, w2)
                            nc.any.tensor_scalar_mul(m1[:], p1[:, i], w1)
                            nc.any.tensor_scalar_mul(m2[:], p2[:, i], w0)
                        else:
                            nc.vector.tensor_scalar_mul(m0[:], x0, w2)
                            nc.vector.tensor_scalar_mul(m1[:], p1[:, i], w1)
                            nc.vector.tensor_scalar_mul(m2[:], p2[:, i], w0)
                        vtt(t[:], m0[:], m1[:], ADD)
                        vtt(ov[:, i], t[:], m2[:], ADD)

            def xyconv(b):
                s = b // _SC
                if s not in osc:
                    osc[s] = opool.tile([128, _SC * _FOUT], f16, tag="osc",
                                        name="osc")
                p = pxypool.tile([128, _FOUT], f32, tag="pxy")
                ds = [d for d in (-1, 0, 1) if 0 <= b + d < _NBLK]
                for ch in range(2):
                    co = ch * 512
                    for i, d in enumerate(ds):
                        nc.tensor.matmul(
                            p[:, co:co + 512], Wt[d + 1],
                            zt[b + d][:, co:co + 512],
                            start=(i == 0), stop=(i == len(ds) - 1))
                lo = (b % _SC) * _FOUT
                nc.scalar.copy(osc[s][:, lo:lo + _FOUT], p[:])
                # Output DMAs ride the ACT DGE queue (half a super-chunk at
                # a time), emitted right after the evac of their second
                # block (same engine, so the wait is already satisfied at
                # dispatch and never blocks the input queue on SP).
                ov = osc[s][:].rearrange("p (f q) -> p f q", f=_SC)
                if b % 2 == 1:
                    f0 = (b % _SC) - 1
                    nc.scalar.dma_start(out=yout_v[s, 0][:, f0:f0 + 2],
                                        in_=ov[0:64, f0:f0 + 2])
                    nc.scalar.dma_start(out=yout_v[s, 1][:, f0:f0 + 2],
                                        in_=ov[64:128, f0:f0 + 2])

            next_xy = 0
            for s in range(_NSC):
                if s > 0:
                    load_sc(s)
                for u in range(s * _SC // 2, (s + 1) * _SC // 2):
                    zconv_unit(u)
                # xy for blocks whose zt[b+1] now exists
                hi = min(s * _SC + _SC - 2, _NBLK - 1) if s < _NSC - 1 \
                    else _NBLK - 1
                while next_xy <= hi:
                    xyconv(next_xy)
                    next_xy += 1
    nc.compile()
    return nc


def kernel(image, kernel, _trace=False):
    from concourse.bass_utils import run_bass_kernel_spmd

    image = np.asarray(image)
    if "nc" not in _CACHE:
        _CACHE["nc"] = _build_nc()
        _CACHE["consts"] = _build_consts()
    nc = _CACHE["nc"]
    wmat, zmat = _CACHE["consts"]
    cm = np.concatenate([wmat, zmat], axis=0)  # [8, 128, 128]
    cm = np.ascontiguousarray(cm.transpose(1, 0, 2).reshape(128, -1),
                              dtype=np.float16)

    img16 = np.ascontiguousarray(image, dtype=np.float16)
    in_maps = []
    for k in range(_NC):
        n, h = k // 2, k % 2
        zlo = h * _ZH - 2
        xin = np.zeros((_X, _Y, _ZP, _C), np.float16)
        s0, s1 = max(0, zlo), min(_Z, zlo + _ZP)
        xin[:, :, s0 - zlo: s1 - zlo, :] = img16[n, :, :, s0:s1, :]
        in_maps.append({"xin": xin, "cm": cm})

    res = run_bass_kernel_spmd(nc, in_maps, list(range(_NC)), trace=_trace)
    out = np.empty((_NB, _X, _Y, _Z, _C), np.float32)
    for k in range(_NC):
        n, h = k // 2, k % 2
        out[n, :, :, h * _ZH:(h + 1) * _ZH, :] = \
            res.results[k]["yout"].astype(np.float32)
    if _trace:
        return out, res
    return out


# revision 17
# speedup vs baseline: 1.0907x; 1.0006x over previous
# Trainium2 Bass kernel: depthwise 3D Gaussian low-pass filter (5x5x5, separable)
# image [4, 64, 64, 64, 32] (n, x, y, z, c) -> same-shape output, stride 1, pad 2.
#
# Sharding: 8 cores = (n, z-half). Each core owns n = k//2, z in [32*(k%2), +32)
# plus a 2-deep z halo each side (zero-padded at volume edges). All I/O fp16.
#
# Per core (partitions p = (y-parity a, x), free = (z, c)):
#   stage A (z-conv, 5-tap, runs FIRST so it consumes the z halo): per y-pair
#     block b, zt[b][p, (z32 c)] = sum_j w_j x[b][p, (z+j, c)], decomposed as
#     symmetric window-adds + scalar muls + combines (2x/4x fp16 DVE modes).
#     Work is spread across engines per 2-block unit (see _ZUNITS): DVE,
#     GpSimd adds or muls, ACT muls, or PE (5 shifted scaled-identity
#     matmuls + ACT evac).  GpSimd only gets tensor_tensor / tensor_scalar
#     (walrus rejects TensorScalarPtr-with-3-inputs on Pool).
#   stage B (xy-conv): out_block b = sum_d W_d.T @ zt[b+d], d in {-1,0,1},
#     where W_d[128,128] = Toeplitz_x (5-tap) x y-parity band; 3 fp16 matmuls
#     per 512-col chunk accumulated in fp32 PSUM, one ACT evac per block.
# DMAs are batched 4 blocks per transfer to amortize DGE overhead.
import numpy as np

_SIGMA = 0.5 * (2.0 ** 2 - 1) ** 0.5  # scale = 2.0
_KS = 5
_NC = 8
_X, _Y, _Z, _C, _NB = 64, 64, 64, 32, 4
_ZH = _Z // 2          # z extent per core (32)
_ZP = _ZH + 4          # with halo (36)
_FIN = _ZP * _C        # 1152 free elements in per block
_FOUT = _ZH * _C       # 1024 free elements out per block
_NBLK = _Y // 2        # 32 y-pair blocks
_SC = 4                # blocks per DMA super-chunk
_NSC = _NBLK // _SC    # 8 super-chunks

# Per-unit (2 consecutive y-pair blocks) z-conv engine assignment, found by
# randomized search against the instruction cost model:
#   P = PE shifted-identity matmuls (+ ACT evac)
#   K = GpSimd window-adds + DVE muls/combines
#   Q = DVE adds/combines + GpSimd muls
#   S = DVE adds/combines + ACT muls
#   D = all DVE
#   F = fast-start unit (per-block minimal-latency chains)
#   A = all nc.any (scheduler gap-fills DVE/ACT)
_ZUNITS = list("PDDPFPKPDPDKDKKD")
assert len(_ZUNITS) == _NBLK // 2
_XBUFS = 5    # input super-chunk buffers
_ZTBUFS = 14  # zt tile buffers
_PXYBUFS = 3  # xy psum buffers
_PZBUFS = 1   # z-conv psum buffers (P units)
_TMPBUFS = 3  # z-conv scratch buffers per tag
_WARMUP = 16  # dummy matmuls to ramp the PE clock during initial DMA

_CACHE = {}


def _wn():
    r = np.arange(_KS, dtype=np.float64) - _KS // 2
    w = np.exp(-(r ** 2) / (2 * _SIGMA ** 2))
    return w / w.sum()


def _build_consts():
    wn = _wn()
    Bx = np.zeros((64, 64))
    for x in range(64):
        for xp in range(max(0, x - 2), min(64, x + 3)):
            Bx[x, xp] = wn[xp - x + 2]
    wmat = np.zeros((3, 128, 128))
    for di, d in enumerate((-1, 0, 1)):
        for a in range(2):
            for a2 in range(2):
                idx = 2 * d + a - a2 + 2
                if 0 <= idx < _KS:
                    wmat[di, a * 64:(a + 1) * 64, a2 * 64:(a2 + 1) * 64] = Bx * wn[idx]
    zmat = np.zeros((_KS, 128, 128))
    for j in range(_KS):
        zmat[j] = np.eye(128) * wn[j]
    return wmat.astype(np.float16), zmat.astype(np.float16)


def _build_nc():
    import concourse.bacc as bacc
    import concourse.mybir as mybir
    import concourse.tile as tile

    f32 = mybir.dt.float32
    f16 = mybir.dt.float16
    ADD = mybir.AluOpType.add
    MUL = mybir.AluOpType.mult

    wnf = _wn()

    nc = bacc.Bacc("TRN2", target_bir_lowering=False, debug=False,
                   num_devices=_NC)
    xin = nc.dram_tensor("xin", [_X, _Y, _ZP, _C], f16, kind="ExternalInput")
    cm = nc.dram_tensor("cm", [128, (3 + _KS) * 128], f16, kind="ExternalInput")
    yout = nc.dram_tensor("yout", [_X, _Y, _ZH, _C], f16, kind="ExternalOutput")

    # [x, y, z, c] -> [s, a, x, (f z c)]: y = 2*(4s+f) + a.  Partition dim is
    # x (64); the two y parities need separate DMAs (a=0 -> partitions 0:64,
    # a=1 -> 64:128) since (a x) strides can't fuse into one AP dim.
    xin_v = xin.ap().rearrange("x (s f a) z c -> s a x f (z c)", f=_SC, a=2)
    yout_v = yout.ap().rearrange("x (s f a) z c -> s a x f (z c)", f=_SC, a=2)

    with tile.TileContext(nc) as tc:
        with (
            tc.tile_pool(name="consts", bufs=1) as cpool,
            tc.tile_pool(name="xsc", bufs=_XBUFS) as xpool,
            tc.tile_pool(name="zt", bufs=_ZTBUFS) as ztpool,
            tc.tile_pool(name="tmp", bufs=_TMPBUFS) as tpool,
            tc.tile_pool(name="osc", bufs=3) as opool,
            tc.tile_pool(name="pxy", bufs=_PXYBUFS, space="PSUM") as pxypool,
            tc.tile_pool(name="pz", bufs=_PZBUFS, space="PSUM") as pzpool,
        ):
            xsc = {}   # super-chunk s -> input tile [128, 4*1152]
            zt = {}    # block b -> z-convolved tile [128, 1024]
            osc = {}   # super-chunk s -> output tile [128, 4*1024]

            def load_sc(s, split=False):
                t = xpool.tile([128, _SC * _FIN], f16, tag="xsc")
                tv = t[:].rearrange("p (f q) -> p f q", f=_SC)
                if split:
                    # First chunk: land blocks 0-1 first so the z-conv
                    # pipeline starts ~1.6us earlier.
                    h = _SC // 2
                    nc.sync.dma_start(out=tv[0:64, 0:h], in_=xin_v[s, 0][:, 0:h])
                    nc.sync.dma_start(out=tv[64:128, 0:h],
                                      in_=xin_v[s, 1][:, 0:h])
                    nc.sync.dma_start(out=tv[0:64, h:_SC],
                                      in_=xin_v[s, 0][:, h:_SC])
                    nc.sync.dma_start(out=tv[64:128, h:_SC],
                                      in_=xin_v[s, 1][:, h:_SC])
                else:
                    nc.sync.dma_start(out=tv[0:64], in_=xin_v[s, 0])
                    nc.sync.dma_start(out=tv[64:128], in_=xin_v[s, 1])
                xsc[s] = t

            # All 8 [128,128] weight matrices in one DMA; cm is
            # pre-transposed host-side to [p, (t f)] so the transfer is 128
            # fully-contiguous 2KB runs (sub-512B runs pay a 2x DMA penalty).
            cmt = cpool.tile([128, (3 + _KS) * 128], f16, tag="cmt")
            nc.sync.dma_start(out=cmt[:], in_=cm.ap()[:, :])
            load_sc(0)
            Wt = [cmt[:, di * 128:(di + 1) * 128] for di in range(3)]
            Zt = [cmt[:, (3 + j) * 128:(4 + j) * 128] for j in range(_KS)]

            w0, w1, w2 = float(wnf[0]), float(wnf[1]), float(wnf[2])

            if _WARMUP:
                # Ramp the PE clock while the first input DMA is in flight;
                # results are discarded (overwritten by the first P-unit).
                pw = pzpool.tile([128, _FOUT], f32, tag="pz", name="pz")
                for _ in range(_WARMUP):
                    nc.tensor.matmul(pw[:, 0:128], Zt[0], cmt[:, 0:128],
                                     start=True, stop=True)

            def zconv_unit(u):
                # z-conv for blocks 2u, 2u+1 as [128, 2, 1024] batched ops
                kind = _ZUNITS[u]
                b0 = 2 * u
                x = xsc[b0 // _SC]
                xv = x[:].rearrange("p (f q) -> p f q", f=_SC)
                fu = (b0 % _SC)  # first block's index within the super-chunk

                def xs(j):
                    # tap-j shifted [128, 2, 1024] view of the unit's input
                    return xv[:, fu:fu + 2, j * _C: j * _C + _FOUT]

                o = ztpool.tile([128, 2 * _FOUT], f16, tag="zt")
                ov = o[:].rearrange("p (f q) -> p f q", f=2)
                zt[b0] = o[:, 0:_FOUT]
                zt[b0 + 1] = o[:, _FOUT:2 * _FOUT]
                if kind == "P":
                    for i in range(2):
                        p = pzpool.tile([128, _FOUT], f32, tag="pz")
                        for ch in range(2):
                            co = ch * 512
                            for j in range(_KS):
                                nc.tensor.matmul(
                                    p[:, co:co + 512], Zt[j],
                                    xv[:, fu + i, j * _C + co: j * _C + co + 512],
                                    start=(j == 0), stop=(j == _KS - 1))
                        nc.scalar.copy(ov[:, i], p[:])
                elif kind == "F":
                    # Fast-start unit: fully per-block, minimal-latency DVE
                    # chain for block 0; block 1 gets GpSimd window-adds so
                    # both zts land ~simultaneously right after the first DMA.
                    for i in range(2):
                        p1 = tpool.tile([128, _FOUT], f16, tag="p1", name="p1")
                        p2 = tpool.tile([128, _FOUT], f16, tag="p2", name="p2")
                        m0 = tpool.tile([128, _FOUT], f16, tag="m0", name="m0")
                        m1 = tpool.tile([128, _FOUT], f16, tag="m1", name="m1")
                        t = tpool.tile([128, _FOUT], f16, tag="tt", name="tt")
                        lo = (fu + i) * _FIN
                        def xsl(j, lo=lo):
                            return x[:, lo + j * _C: lo + j * _C + _FOUT]
                        eng = nc.vector if i == 0 else nc.gpsimd
                        eng.tensor_tensor(p1[:], xsl(1), xsl(3), ADD)
                        eng.tensor_tensor(p2[:], xsl(0), xsl(4), ADD)
                        nc.vector.tensor_scalar_mul(m0[:], xsl(2), w2)
                        nc.vector.tensor_scalar_mul(m1[:], p1[:], w1)
                        nc.vector.tensor_tensor(t[:], m0[:], m1[:], ADD)
                        nc.vector.tensor_scalar_mul(m1[:], p2[:], w0)
                        nc.vector.tensor_tensor(ov[:, i], t[:], m1[:], ADD)
                elif kind == "K":
                    # GpSimd: window-adds p1, p2 (per block, 1D APs);
                    # DVE: muls + combines (batched)
                    p1t = tpool.tile([128, 2 * _FOUT], f16, tag="p1", name="p1")
                    p2t = tpool.tile([128, 2 * _FOUT], f16, tag="p2", name="p2")
                    m0t = tpool.tile([128, 2 * _FOUT], f16, tag="m0", name="m0")
                    m1t = tpool.tile([128, 2 * _FOUT], f16, tag="m1", name="m1")
                    ttt = tpool.tile([128, 2 * _FOUT], f16, tag="tt", name="tt")
                    for i in range(2):
                        lo = (fu + i) * _FIN
                        sl = slice(i * _FOUT, (i + 1) * _FOUT)
                        nc.gpsimd.tensor_tensor(
                            p1t[:, sl], x[:, lo + _C:lo + _C + _FOUT],
                            x[:, lo + 3 * _C:lo + 3 * _C + _FOUT], ADD)
                        nc.gpsimd.tensor_tensor(
                            p2t[:, sl], x[:, lo:lo + _FOUT],
                            x[:, lo + 4 * _C:lo + 4 * _C + _FOUT], ADD)
                    p1 = p1t[:].rearrange("p (f q) -> p f q", f=2)
                    p2 = p2t[:].rearrange("p (f q) -> p f q", f=2)
                    m0 = m0t[:].rearrange("p (f q) -> p f q", f=2)
                    m1 = m1t[:].rearrange("p (f q) -> p f q", f=2)
                    t = ttt[:].rearrange("p (f q) -> p f q", f=2)
                    nc.vector.tensor_scalar_mul(m0, xs(2), w2)
                    nc.vector.tensor_scalar_mul(m1, p1, w1)
                    nc.vector.tensor_tensor(t, m0, m1, ADD)
                    m2 = m0  # reuse
                    nc.vector.tensor_scalar_mul(m2, p2, w0)
                    nc.vector.tensor_tensor(ov, t, m2, ADD)
                else:  # Q / S / D / A: adds batched, muls+combines per block
                    p1t = tpool.tile([128, 2 * _FOUT], f16, tag="p1", name="p1")
                    p2t = tpool.tile([128, 2 * _FOUT], f16, tag="p2", name="p2")
                    p1 = p1t[:].rearrange("p (f q) -> p f q", f=2)
                    p2 = p2t[:].rearrange("p (f q) -> p f q", f=2)
                    vtt = nc.any.tensor_tensor if kind == "A" else \
                        nc.vector.tensor_tensor
                    vtt(p1, xs(1), xs(3), ADD)
                    vtt(p2, xs(0), xs(4), ADD)
                    for i in range(2):
                        m0 = tpool.tile([128, _FOUT], f16, tag="m0", name="m0")
                        m1 = tpool.tile([128, _FOUT], f16, tag="m1", name="m1")
                        m2 = tpool.tile([128, _FOUT], f16, tag="m2", name="m2")
                        t = tpool.tile([128, _FOUT], f16, tag="tt", name="tt")
                        x0 = xv[:, fu + i, 2 * _C: 2 * _C + _FOUT]
                        if kind == "Q":
                            nc.gpsimd.tensor_scalar_mul(m0[:], x0, w2)
                            nc.gpsimd.tensor_scalar_mul(m1[:], p1[:, i], w1)
                            nc.gpsimd.tensor_scalar_mul(m2[:], p2[:, i], w0)
                        elif kind == "S":
                            nc.scalar.mul(m0[:], x0, w2)
                            nc.scalar.mul(m1[:], p1[:, i], w1)
                            nc.scalar.mul(m2[:], p2[:, i], w0)
                        elif kind == "A":
                            nc.any.tensor_scalar_mul(m0[:], x0, w2)
                            nc.any.tensor_scalar_mul(m1[:], p1[:, i], w1)
                            nc.any.tensor_scalar_mul(m2[:], p2[:, i], w0)
                        else:
                            nc.vector.tensor_scalar_mul(m0[:], x0, w2)
                            nc.vector.tensor_scalar_mul(m1[:], p1[:, i], w1)
                            nc.vector.tensor_scalar_mul(m2[:], p2[:, i], w0)
                        vtt(t[:], m0[:], m1[:], ADD)
                        vtt(ov[:, i], t[:], m2[:], ADD)

            def xyconv(b):
                s = b // _SC
                if s not in osc:
                    osc[s] = opool.tile([128, _SC * _FOUT], f16, tag="osc",
                                        name="osc")
                p = pxypool.tile([128, _FOUT], f32, tag="pxy")
                ds = [d for d in (-1, 0, 1) if 0 <= b + d < _NBLK]
                for ch in range(2):
                    co = ch * 512
                    for i, d in enumerate(ds):
                        nc.tensor.matmul(
                            p[:, co:co + 512], Wt[d + 1],
                            zt[b + d][:, co:co + 512],
                            start=(i == 0), stop=(i == len(ds) - 1))
                lo = (b % _SC) * _FOUT
                nc.scalar.copy(osc[s][:, lo:lo + _FOUT], p[:])
                # Output DMAs ride the ACT DGE queue (half a super-chunk at
                # a time), emitted right after the evac of their second
                # block (same engine, so the wait is already satisfied at
                # dispatch and never blocks the input queue on SP).
                ov = osc[s][:].rearrange("p (f q) -> p f q", f=_SC)
                if b % 2 == 1:
                    f0 = (b % _SC) - 1
                    nc.scalar.dma_start(out=yout_v[s, 0][:, f0:f0 + 2],
                                        in_=ov[0:64, f0:f0 + 2])
                    nc.scalar.dma_start(out=yout_v[s, 1][:, f0:f0 + 2],
                                        in_=ov[64:128, f0:f0 + 2])

            next_xy = 0
            for s in range(_NSC):
                if s > 0:
                    load_sc(s)
                for u in range(s * _SC // 2, (s + 1) * _SC // 2):
                    zconv_unit(u)
                # xy for blocks whose zt[b+1] now exists
                hi = min(s * _SC + _SC - 2, _NBLK - 1) if s < _NSC - 1 \
                    else _NBLK - 1
                while next_xy <= hi:
                    xyconv(next_xy)
                    next_xy += 1
    nc.compile()
    return nc


def kernel(image, kernel, _trace=False):
    from concourse.bass_utils import run_bass_kernel_spmd

    image = np.asarray(image)
    if "nc" not in _CACHE:
        _CACHE["nc"] = _build_nc()
        _CACHE["consts"] = _build_consts()
    nc = _CACHE["nc"]
    wmat, zmat = _CACHE["consts"]
    cm = np.concatenate([wmat, zmat], axis=0)  # [8, 128, 128]
    cm = np.ascontiguousarray(cm.transpose(1, 0, 2).reshape(128, -1),
                              dtype=np.float16)

    img16 = np.ascontiguousarray(image, dtype=np.float16)
    in_maps = []
    for k in range(_NC):
        n, h = k // 2, k % 2
        zlo = h * _ZH - 2
        xin = np.zeros((_X, _Y, _ZP, _C), np.float16)
        s0, s1 = max(0, zlo), min(_Z, zlo + _ZP)
        xin[:, :, s0 - zlo: s1 - zlo, :] = img16[n, :, :, s0:s1, :]
        in_maps.append({"xin": xin, "cm": cm})

    res = run_bass_kernel_spmd(nc, in_maps, list(range(_NC)), trace=_trace)
    out = np.empty((_NB, _X, _Y, _Z, _C), np.float32)
    for k in range(_NC):
        n, h = k // 2, k % 2
        out[n, :, :, h * _ZH:(h + 1) * _ZH, :] = \
            res.results[k]["yout"].astype(np.float32)
    if _trace:
        return out, res
    return out
